# revision 14
# baseline (speedup 1.0000x reference)
"""LocalMeanInpainter Trainium2 kernel.

out = x*mask + (box15(x)/box15(ones))*(1-mask)  over (32,3,512,512) f32.

Strategy: data-parallel over batch (4 images x 3 channels = 12 planes of
512x512 per core, 8 cores). Per plane, the 15x15 box mean is separable:
mean = diag(1/ch) @ B @ X @ B @ diag(1/cw) with B the 0/1 banded matrix
(|i-j|<=7) and ch/cw the 1-D in-bounds counts (cnt = outer(ch, cw) exactly).
Both passes run on the PE tensor engine with the normalization folded into
the B weights:
  pass1: S1T[w, h_out] = sum_h X[h, w] * BH[h, h_out]   (X chunk stationary)
  pass2: S2[h_out, w_out] = sum_w S1T[w, h_out] * BW[w, w_out]
Blend: mask is exactly {0,1}, so out = select(mask, x, mean): one DVE
copy_predicated that overwrites mean (in PSUM) with x where mask!=0, then
DMA straight from PSUM to DRAM.
"""

import numpy as np
import ml_dtypes

H = 512
W = 512
WINDOW = 15
PAD = 7
N_CORES = 8
IMGS_PER_CORE = 4
CHANNELS = 3
PLANES = IMGS_PER_CORE * CHANNELS  # 12
NCHUNK = H // 128  # 4

_CACHE = {}


def _band_matrix(n, normalize_cols):
    idx = np.arange(n)
    band = (np.abs(idx[:, None] - idx[None, :]) <= PAD).astype(np.float64)
    if normalize_cols:
        cnt = np.minimum(idx + PAD, n - 1) - np.maximum(idx - PAD, 0) + 1
        band = band / cnt[None, :]
    return band


def _build_program(planes=PLANES, reps=1):
    import concourse.bass as bass
    import concourse.tile as tile
    from concourse import bacc, mybir

    f32 = mybir.dt.float32
    bf16 = mybir.dt.bfloat16

    nc = bacc.Bacc("TRN2", target_bir_lowering=False, debug=False, num_devices=N_CORES)
    x_d = nc.declare_dram_parameter("x", [planes, H, W], f32, isOutput=False)
    m_d = nc.declare_dram_parameter("mask", [planes, H, W], mybir.dt.uint8, isOutput=False)
    bh_d = nc.declare_dram_parameter("bh", [H, H], bf16, isOutput=False)
    bw_d = nc.declare_dram_parameter("bw", [W, W], bf16, isOutput=False)
    out_d = nc.declare_dram_parameter("out", [planes, H, W], f32, isOutput=True)

    with tile.TileContext(nc) as tc:
        with (
            tc.tile_pool(name="consts", bufs=1) as cpool,
            tc.tile_pool(name="xt", bufs=4) as xpool,
            tc.tile_pool(name="mt", bufs=4) as mpool,
            tc.tile_pool(name="xb", bufs=3) as xbpool,
            tc.tile_pool(name="s1b", bufs=3) as s1pool,
            tc.tile_pool(name="ot", bufs=6) as opool,
            tc.tile_pool(name="ps1", bufs=3, space="PSUM") as ps1pool,
            tc.tile_pool(name="ps2", bufs=5, space="PSUM") as ps2pool,
        ):
            # B constants: stored [128, (chunk, 512)] — partition = row within
            # chunk, free slice c selects row-chunk c.
            bh_t = cpool.tile([128, NCHUNK * H], bf16, tag="bh")
            nc.sync.dma_start(
                out=bh_t[:].rearrange("h (c n) -> h c n", c=NCHUNK),
                in_=bh_d[:].rearrange("(c h) n -> h c n", c=NCHUNK),
            )
            bw_t = cpool.tile([128, NCHUNK * W], bf16, tag="bw")
            nc.sync.dma_start(
                out=bw_t[:].rearrange("h (c n) -> h c n", c=NCHUNK),
                in_=bw_d[:].rearrange("(c h) n -> h c n", c=NCHUNK),
            )

            from contextlib import nullcontext

            loop_ctx = (
                tc.For_i(
                    0,
                    reps,
                    1,
                    hint_engines=tuple(
                        getattr(mybir.EngineType, e) for e in ("PE", "Activation", "DVE", "SP", "Pool")
                    ),
                )
                if reps > 1
                else nullcontext()
            )
            with loop_ctx:
              for p in range(planes):
                xt = xpool.tile([128, NCHUNK * W], f32, tag="xt")
                nc.sync.dma_start(
                    out=xt[:].rearrange("h (c w) -> h c w", c=NCHUNK),
                    in_=x_d[p].rearrange("(c h) w -> h c w", c=NCHUNK),
                )
                mt = mpool.tile([128, NCHUNK * W], mybir.dt.uint8, tag="mt")
                nc.sync.dma_start(
                    out=mt[:].rearrange("h (c w) -> h c w", c=NCHUNK),
                    in_=m_d[p].rearrange("(c h) w -> h c w", c=NCHUNK),
                )
                xb = xbpool.tile([128, NCHUNK * W], bf16, tag="xb")
                for c in range(NCHUNK):
                    nc.scalar.copy(
                        xb[:, c * W : (c + 1) * W], xt[:, c * W : (c + 1) * W]
                    )

                # Banded accumulation: contraction chunk kc only touches
                # output columns [128k-7, 128k+135). Split into an exclusive
                # segment (start=True) and 14-wide boundary overlaps written
                # by two adjacent chunks (first start=True, second accumulates).
                def banded_mms(ps, lhsT_of, rhs_tile, rhs_base):
                    for kc in range(NCHUNK):
                        lo, hi = 128 * kc, 128 * (kc + 1)
                        segs = []
                        if kc > 0:
                            segs.append((lo - PAD, lo + PAD, False, True))
                        e0 = lo if kc == 0 else lo + PAD
                        e1 = hi if kc == NCHUNK - 1 else hi - PAD
                        segs.append((e0, e1, True, True))
                        if kc < NCHUNK - 1:
                            segs.append((hi - PAD, hi + PAD, True, False))
                        lhsT = lhsT_of(kc)
                        for c0, c1, st, sp in segs:
                            nc.tensor.matmul(
                                ps[:, c0:c1],
                                lhsT=lhsT,
                                rhs=rhs_tile[:, rhs_base(kc) + c0 : rhs_base(kc) + c1],
                                start=st,
                                stop=sp,
                            )

                # pass 1: S1T[wc] [128 w, 512 h_out] accumulated over h chunks
                s1b = s1pool.tile([128, NCHUNK * H], bf16, tag="s1b")
                for wc in range(NCHUNK):
                    ps1 = ps1pool.tile([128, H], f32, tag="ps1")
                    banded_mms(
                        ps1,
                        lambda kc: xb[:, kc * W + wc * 128 : kc * W + wc * 128 + 128],
                        bh_t,
                        lambda kc: kc * H,
                    )
                    nc.scalar.copy(s1b[:, wc * H : (wc + 1) * H], ps1[:])

                # pass 2: S2[mc] [128 h_out, 512 w_out] accumulated over w chunks
                for mc in range(NCHUNK):
                    ps2 = ps2pool.tile([128, W], f32, tag="ps2")
                    banded_mms(
                        ps2,
                        lambda kc: s1b[:, kc * H + mc * 128 : kc * H + mc * 128 + 128],
                        bw_t,
                        lambda kc: kc * W,
                    )
                    # blend: keep mean where mask==0, overwrite with x where 1
                    ot = opool.tile([128, W], f32, tag="ot")
                    nc.vector.tensor_copy(ot[:], ps2[:])
                    nc.vector.copy_predicated(
                        ot[:],
                        mt[:, mc * W : (mc + 1) * W],
                        xt[:, mc * W : (mc + 1) * W],
                    )
                    nc.sync.dma_start(
                        out=out_d[p, mc * 128 : (mc + 1) * 128, :], in_=ot[:]
                    )
    nc.finalize()
    return nc


def _get_program():
    if "nc" not in _CACHE:
        _CACHE["nc"] = _build_program()
        _CACHE["bh"] = _band_matrix(H, True).astype(ml_dtypes.bfloat16)
        _CACHE["bw"] = _band_matrix(W, True).astype(ml_dtypes.bfloat16)
    return _CACHE["nc"], _CACHE["bh"], _CACHE["bw"]


def kernel(x: np.ndarray, mask: np.ndarray) -> np.ndarray:
    from concourse.bass_utils import run_bass_kernel_spmd

    nc, bh, bw = _get_program()

    x = np.ascontiguousarray(x, dtype=np.float32)
    mask = np.ascontiguousarray(mask).astype(np.uint8)
    xs = x.reshape(N_CORES, PLANES, H, W)
    ms = mask.reshape(N_CORES, PLANES, H, W)

    in_maps = [
        {"x": xs[i], "mask": ms[i], "bh": bh, "bw": bw} for i in range(N_CORES)
    ]
    res = run_bass_kernel_spmd(nc, in_maps, core_ids=list(range(N_CORES)))
    out = np.stack([res.results[i]["out"] for i in range(N_CORES)])
    return out.reshape(x.shape[0] // IMGS_PER_CORE, IMGS_PER_CORE, CHANNELS, H, W).reshape(
        -1, CHANNELS, H, W
    )


# revision 17
# speedup vs baseline: 1.0626x; 1.0626x over previous
"""LocalMeanInpainter Trainium2 kernel.

out = x*mask + (box15(x)/box15(ones))*(1-mask)  over (32,3,512,512) f32.

Strategy: data-parallel over batch (4 images x 3 channels = 12 planes of
512x512 per core, 8 cores). Per plane, the 15x15 box mean is separable:
mean = diag(1/ch) @ B @ X @ B @ diag(1/cw) with B the 0/1 banded matrix
(|i-j|<=7) and ch/cw the 1-D in-bounds counts (cnt = outer(ch, cw) exactly).
Both passes run on the PE tensor engine with the normalization folded into
the B weights:
  pass1: S1T[w, h_out] = sum_h X[h, w] * BH[h, h_out]   (X chunk stationary)
  pass2: S2[h_out, w_out] = sum_w S1T[w, h_out] * BW[w, w_out]
Blend: mask is exactly {0,1} (shipped as uint8), so out = select(mask, x,
mean): DVE tensor_copy from PSUM + copy_predicated, then DMA out.
"""

import numpy as np
import ml_dtypes

H = 512
W = 512
WINDOW = 15
PAD = 7
N_CORES = 8
IMGS_PER_CORE = 4
CHANNELS = 3
PLANES = IMGS_PER_CORE * CHANNELS  # 12
NCHUNK = H // 128  # 4

# matmul operand dtype: "f32r" = fp32 bits in the PE's full-rate replicated
# mode (no x cast needed), "bf16" = cast x/S1 to bf16 first.
MM_DTYPE = "f32r"
BANDED = False

_CACHE = {}


def _band_matrix(n, normalize_cols):
    idx = np.arange(n)
    band = (np.abs(idx[:, None] - idx[None, :]) <= PAD).astype(np.float64)
    if normalize_cols:
        cnt = np.minimum(idx + PAD, n - 1) - np.maximum(idx - PAD, 0) + 1
        band = band / cnt[None, :]
    return band


def _build_program(planes=PLANES, reps=1, mm_dtype=None, banded=None):
    import concourse.tile as tile
    from concourse import bacc, mybir
    from contextlib import nullcontext

    if mm_dtype is None:
        mm_dtype = MM_DTYPE
    if banded is None:
        banded = BANDED
    f32 = mybir.dt.float32
    use_f32r = mm_dtype == "f32r"
    w_dt = mybir.dt.float32r if use_f32r else mybir.dt.bfloat16
    s1_dt = mybir.dt.float32r if use_f32r else mybir.dt.bfloat16

    nc = bacc.Bacc("TRN2", target_bir_lowering=False, debug=False, num_devices=N_CORES)
    x_dt = mybir.dt.float32r if use_f32r else f32
    x_d = nc.declare_dram_parameter("x", [planes, H, W], x_dt, isOutput=False)
    m_d = nc.declare_dram_parameter(
        "mask", [planes, H, W], mybir.dt.uint8, isOutput=False
    )
    bh_d = nc.declare_dram_parameter("bh", [H, H], w_dt, isOutput=False)
    bw_d = nc.declare_dram_parameter("bw", [W, W], w_dt, isOutput=False)
    out_d = nc.declare_dram_parameter("out", [planes, H, W], f32, isOutput=True)

    def as_mm(ap):
        return ap.bitcast(mybir.dt.float32r) if use_f32r else ap

    with tile.TileContext(nc) as tc:
        with (
            tc.tile_pool(name="consts", bufs=1) as cpool,
            tc.tile_pool(name="xt", bufs=4) as xpool,
            tc.tile_pool(name="mt", bufs=4) as mpool,
            tc.tile_pool(name="xb", bufs=3) as xbpool,
            tc.tile_pool(name="s1b", bufs=3) as s1pool,
            tc.tile_pool(name="ot", bufs=6) as opool,
            tc.tile_pool(name="ps1", bufs=3, space="PSUM") as ps1pool,
            tc.tile_pool(name="ps2", bufs=5, space="PSUM") as ps2pool,
        ):
            # B constants: stored [128, (chunk, 512)] — partition = row within
            # chunk, free slice c selects row-chunk c.
            bh_t = cpool.tile([128, NCHUNK * H], w_dt, tag="bh")
            nc.sync.dma_start(
                out=bh_t[:].rearrange("h (c n) -> h c n", c=NCHUNK),
                in_=bh_d[:].rearrange("(c h) n -> h c n", c=NCHUNK),
            )
            bw_t = cpool.tile([128, NCHUNK * W], w_dt, tag="bw")
            nc.sync.dma_start(
                out=bw_t[:].rearrange("h (c n) -> h c n", c=NCHUNK),
                in_=bw_d[:].rearrange("(c h) n -> h c n", c=NCHUNK),
            )

            def mms(ps, lhsT_of, rhs_tile, rhs_base):
                if not banded:
                    for kc in range(NCHUNK):
                        nc.tensor.matmul(
                            ps[:],
                            lhsT=lhsT_of(kc),
                            rhs=rhs_tile[:, rhs_base(kc) : rhs_base(kc) + 512],
                            start=(kc == 0),
                            stop=(kc == NCHUNK - 1),
                        )
                    return
                # Banded: chunk kc only touches output cols [128k-7, 128k+135).
                for kc in range(NCHUNK):
                    lo, hi = 128 * kc, 128 * (kc + 1)
                    segs = []
                    if kc > 0:
                        segs.append((lo - PAD, lo + PAD, False, True))
                    e0 = lo if kc == 0 else lo + PAD
                    e1 = hi if kc == NCHUNK - 1 else hi - PAD
                    segs.append((e0, e1, True, True))
                    if kc < NCHUNK - 1:
                        segs.append((hi - PAD, hi + PAD, True, False))
                    lhsT = lhsT_of(kc)
                    for c0, c1, st, sp in segs:
                        nc.tensor.matmul(
                            ps[:, c0:c1],
                            lhsT=lhsT,
                            rhs=rhs_tile[:, rhs_base(kc) + c0 : rhs_base(kc) + c1],
                            start=st,
                            stop=sp,
                        )

            loop_ctx = (
                tc.For_i(
                    0,
                    reps,
                    1,
                    hint_engines=tuple(
                        getattr(mybir.EngineType, e)
                        for e in ("PE", "Activation", "DVE", "SP", "Pool")
                    ),
                )
                if reps > 1
                else nullcontext()
            )
            with loop_ctx:
              for p in range(planes):
                xt = xpool.tile([128, NCHUNK * W], x_dt, tag="xt")
                nc.sync.dma_start(
                    out=xt[:].rearrange("h (c w) -> h c w", c=NCHUNK),
                    in_=x_d[p].rearrange("(c h) w -> h c w", c=NCHUNK),
                )
                mt = mpool.tile([128, NCHUNK * W], mybir.dt.uint8, tag="mt")
                nc.sync.dma_start(
                    out=mt[:].rearrange("h (c w) -> h c w", c=NCHUNK),
                    in_=m_d[p].rearrange("(c h) w -> h c w", c=NCHUNK),
                )
                if use_f32r:
                    xmm = xt
                else:
                    xmm = xbpool.tile([128, NCHUNK * W], s1_dt, tag="xb")
                    for c in range(NCHUNK):
                        nc.scalar.copy(
                            xmm[:, c * W : (c + 1) * W], xt[:, c * W : (c + 1) * W]
                        )

                # pass 1: S1T[wc] [128 w, 512 h_out] accumulated over h chunks
                s1b = s1pool.tile([128, NCHUNK * H], s1_dt, tag="s1b")
                for wc in range(NCHUNK):
                    ps1 = ps1pool.tile([128, H], f32, tag="ps1")
                    mms(
                        ps1,
                        lambda kc: as_mm(
                            xmm[:, kc * W + wc * 128 : kc * W + wc * 128 + 128]
                        ),
                        bh_t,
                        lambda kc: kc * H,
                    )
                    nc.scalar.copy(s1b[:, wc * H : (wc + 1) * H], ps1[:])

                # pass 2: S2[mc] [128 h_out, 512 w_out] accumulated over w chunks
                for mc in range(NCHUNK):
                    ps2 = ps2pool.tile([128, W], f32, tag="ps2")
                    mms(
                        ps2,
                        lambda kc: as_mm(
                            s1b[:, kc * H + mc * 128 : kc * H + mc * 128 + 128]
                        ),
                        bw_t,
                        lambda kc: kc * W,
                    )
                    # blend: keep mean where mask==0, overwrite with x where 1
                    ot = opool.tile([128, W], f32, tag="ot")
                    nc.vector.tensor_copy(ot[:], ps2[:])
                    nc.vector.copy_predicated(
                        ot[:],
                        mt[:, mc * W : (mc + 1) * W],
                        xt[:, mc * W : (mc + 1) * W].bitcast(f32),
                    )
                    nc.sync.dma_start(
                        out=out_d[p, mc * 128 : (mc + 1) * 128, :], in_=ot[:]
                    )
    nc.finalize()
    return nc


def _host_weights(mm_dtype=None):
    if mm_dtype is None:
        mm_dtype = MM_DTYPE
    wt = np.float32 if mm_dtype == "f32r" else ml_dtypes.bfloat16
    return (
        _band_matrix(H, True).astype(wt),
        _band_matrix(W, True).astype(wt),
    )


def _get_program():
    if "nc" not in _CACHE:
        _CACHE["nc"] = _build_program()
        _CACHE["bh"], _CACHE["bw"] = _host_weights()
    return _CACHE["nc"], _CACHE["bh"], _CACHE["bw"]


def kernel(x: np.ndarray, mask: np.ndarray) -> np.ndarray:
    from concourse.bass_utils import run_bass_kernel_spmd

    nc, bh, bw = _get_program()

    x = np.ascontiguousarray(x, dtype=np.float32)
    mask = np.ascontiguousarray(mask).astype(np.uint8)
    xs = x.reshape(N_CORES, PLANES, H, W)
    ms = mask.reshape(N_CORES, PLANES, H, W)

    in_maps = [
        {"x": xs[i], "mask": ms[i], "bh": bh, "bw": bw} for i in range(N_CORES)
    ]
    res = run_bass_kernel_spmd(nc, in_maps, core_ids=list(range(N_CORES)))
    out = np.stack([res.results[i]["out"] for i in range(N_CORES)])
    return out.reshape(x.shape[0] // IMGS_PER_CORE, IMGS_PER_CORE, CHANNELS, H, W).reshape(
        -1, CHANNELS, H, W
    )


# revision 18
# speedup vs baseline: 1.0940x; 1.0295x over previous
"""LocalMeanInpainter Trainium2 kernel.

out = x*mask + (box15(x)/box15(ones))*(1-mask)  over (32,3,512,512) f32.

Strategy: data-parallel over batch (4 images x 3 channels = 12 planes of
512x512 per core, 8 cores). Per plane, the 15x15 box mean is separable:
mean = diag(1/ch) @ B @ X @ B @ diag(1/cw) with B the 0/1 banded matrix
(|i-j|<=7) and ch/cw the 1-D in-bounds counts (cnt = outer(ch, cw) exactly).
Both passes run on the PE tensor engine with the normalization folded into
the B weights:
  pass1: S1T[w, h_out] = sum_h X[h, w] * BH[h, h_out]   (X chunk stationary)
  pass2: S2[h_out, w_out] = sum_w S1T[w, h_out] * BW[w, w_out]
Blend: mask is exactly {0,1} (shipped as uint8), so out = select(mask, x,
mean): DVE tensor_copy from PSUM + copy_predicated, then DMA out.
"""

import numpy as np
import ml_dtypes

H = 512
W = 512
WINDOW = 15
PAD = 7
N_CORES = 8
IMGS_PER_CORE = 4
CHANNELS = 3
PLANES = IMGS_PER_CORE * CHANNELS  # 12
NCHUNK = H // 128  # 4

# matmul operand dtype: "f32r" = fp32 bits in the PE's full-rate replicated
# mode (no x cast needed), "bf16" = cast x/S1 to bf16 first.
MM_DTYPE = "f32r"
BANDED = False

_CACHE = {}


def _band_matrix(n, normalize_cols):
    idx = np.arange(n)
    band = (np.abs(idx[:, None] - idx[None, :]) <= PAD).astype(np.float64)
    if normalize_cols:
        cnt = np.minimum(idx + PAD, n - 1) - np.maximum(idx - PAD, 0) + 1
        band = band / cnt[None, :]
    return band


def _build_program(planes=PLANES, reps=1, mm_dtype=None, banded=None):
    import concourse.tile as tile
    from concourse import bacc, mybir
    from contextlib import nullcontext

    if mm_dtype is None:
        mm_dtype = MM_DTYPE
    if banded is None:
        banded = BANDED
    f32 = mybir.dt.float32
    use_f32r = mm_dtype == "f32r"
    w_dt = mybir.dt.float32r if use_f32r else mybir.dt.bfloat16
    s1_dt = mybir.dt.float32r if use_f32r else mybir.dt.bfloat16

    nc = bacc.Bacc("TRN2", target_bir_lowering=False, debug=False, num_devices=N_CORES)
    x_dt = mybir.dt.float32r if use_f32r else f32
    x_d = nc.declare_dram_parameter("x", [planes, H, W], x_dt, isOutput=False)
    m_d = nc.declare_dram_parameter(
        "mask", [planes, H, W], mybir.dt.uint8, isOutput=False
    )
    bh_d = nc.declare_dram_parameter("bh", [H, H], w_dt, isOutput=False)
    bw_d = nc.declare_dram_parameter("bw", [W, W], w_dt, isOutput=False)
    out_d = nc.declare_dram_parameter("out", [planes, H, W], f32, isOutput=True)

    def as_mm(ap):
        return ap.bitcast(mybir.dt.float32r) if use_f32r else ap

    with tile.TileContext(nc) as tc:
        with (
            tc.tile_pool(name="consts", bufs=1) as cpool,
            tc.tile_pool(name="xt", bufs=4) as xpool,
            tc.tile_pool(name="mt", bufs=4) as mpool,
            tc.tile_pool(name="xb", bufs=3) as xbpool,
            tc.tile_pool(name="s1b", bufs=3) as s1pool,
            tc.tile_pool(name="ot", bufs=6) as opool,
            tc.tile_pool(name="ps1", bufs=3, space="PSUM") as ps1pool,
            tc.tile_pool(name="ps2", bufs=5, space="PSUM") as ps2pool,
        ):
            # B constants: stored [128, (chunk, 512)] — partition = row within
            # chunk, free slice c selects row-chunk c.
            bh_t = cpool.tile([128, NCHUNK * H], w_dt, tag="bh")
            nc.sync.dma_start(
                out=bh_t[:].rearrange("h (c n) -> h c n", c=NCHUNK),
                in_=bh_d[:].rearrange("(c h) n -> h c n", c=NCHUNK),
            )
            bw_t = cpool.tile([128, NCHUNK * W], w_dt, tag="bw")
            nc.sync.dma_start(
                out=bw_t[:].rearrange("h (c n) -> h c n", c=NCHUNK),
                in_=bw_d[:].rearrange("(c h) n -> h c n", c=NCHUNK),
            )

            def mms(ps, lhsT_of, rhs_tile, rhs_base):
                if not banded:
                    for kc in range(NCHUNK):
                        nc.tensor.matmul(
                            ps[:],
                            lhsT=lhsT_of(kc),
                            rhs=rhs_tile[:, rhs_base(kc) : rhs_base(kc) + 512],
                            start=(kc == 0),
                            stop=(kc == NCHUNK - 1),
                        )
                    return
                # Banded: chunk kc only touches output cols [128k-7, 128k+135).
                for kc in range(NCHUNK):
                    lo, hi = 128 * kc, 128 * (kc + 1)
                    segs = []
                    if kc > 0:
                        segs.append((lo - PAD, lo + PAD, False, True))
                    e0 = lo if kc == 0 else lo + PAD
                    e1 = hi if kc == NCHUNK - 1 else hi - PAD
                    segs.append((e0, e1, True, True))
                    if kc < NCHUNK - 1:
                        segs.append((hi - PAD, hi + PAD, True, False))
                    lhsT = lhsT_of(kc)
                    for c0, c1, st, sp in segs:
                        nc.tensor.matmul(
                            ps[:, c0:c1],
                            lhsT=lhsT,
                            rhs=rhs_tile[:, rhs_base(kc) + c0 : rhs_base(kc) + c1],
                            start=st,
                            stop=sp,
                        )

            loop_ctx = (
                tc.For_i(
                    0,
                    reps,
                    1,
                    hint_engines=tuple(
                        getattr(mybir.EngineType, e)
                        for e in ("PE", "Activation", "DVE", "SP", "Pool")
                    ),
                )
                if reps > 1
                else nullcontext()
            )
            with loop_ctx:
              for p in range(planes):
                xt = xpool.tile([128, NCHUNK * W], x_dt, tag="xt")
                nc.sync.dma_start(
                    out=xt[:].rearrange("h (c w) -> h c w", c=NCHUNK),
                    in_=x_d[p].rearrange("(c h) w -> h c w", c=NCHUNK),
                )
                mt = mpool.tile([128, NCHUNK * W], mybir.dt.uint8, tag="mt")
                nc.sync.dma_start(
                    out=mt[:].rearrange("h (c w) -> h c w", c=NCHUNK),
                    in_=m_d[p].rearrange("(c h) w -> h c w", c=NCHUNK),
                )
                if use_f32r:
                    xmm = xt
                else:
                    xmm = xbpool.tile([128, NCHUNK * W], s1_dt, tag="xb")
                    for c in range(NCHUNK):
                        nc.scalar.copy(
                            xmm[:, c * W : (c + 1) * W], xt[:, c * W : (c + 1) * W]
                        )

                # pass 1: S1T[wc] [128 w, 512 h_out] accumulated over h chunks
                s1b = s1pool.tile([128, NCHUNK * H], s1_dt, tag="s1b")
                for wc in range(NCHUNK):
                    ps1 = ps1pool.tile([128, H], f32, tag="ps1")
                    mms(
                        ps1,
                        lambda kc: as_mm(
                            xmm[:, kc * W + wc * 128 : kc * W + wc * 128 + 128]
                        ),
                        bh_t,
                        lambda kc: kc * H,
                    )
                    nc.scalar.copy(s1b[:, wc * H : (wc + 1) * H], ps1[:])

                # pass 2: S2[mc] [128 h_out, 512 w_out] accumulated over w chunks
                for mc in range(NCHUNK):
                    ps2 = ps2pool.tile([128, W], f32, tag="ps2")
                    mms(
                        ps2,
                        lambda kc: as_mm(
                            s1b[:, kc * H + mc * 128 : kc * H + mc * 128 + 128]
                        ),
                        bw_t,
                        lambda kc: kc * W,
                    )
                    # blend: keep mean where mask==0, overwrite with x where 1
                    ot = opool.tile([128, W], f32, tag="ot")
                    nc.scalar.copy(ot[:], ps2[:])
                    nc.vector.copy_predicated(
                        ot[:],
                        mt[:, mc * W : (mc + 1) * W],
                        xt[:, mc * W : (mc + 1) * W].bitcast(f32),
                    )
                    nc.sync.dma_start(
                        out=out_d[p, mc * 128 : (mc + 1) * 128, :], in_=ot[:]
                    )
    nc.finalize()
    return nc


def _host_weights(mm_dtype=None):
    if mm_dtype is None:
        mm_dtype = MM_DTYPE
    wt = np.float32 if mm_dtype == "f32r" else ml_dtypes.bfloat16
    return (
        _band_matrix(H, True).astype(wt),
        _band_matrix(W, True).astype(wt),
    )


def _get_program():
    if "nc" not in _CACHE:
        _CACHE["nc"] = _build_program()
        _CACHE["bh"], _CACHE["bw"] = _host_weights()
    return _CACHE["nc"], _CACHE["bh"], _CACHE["bw"]


def kernel(x: np.ndarray, mask: np.ndarray) -> np.ndarray:
    from concourse.bass_utils import run_bass_kernel_spmd

    nc, bh, bw = _get_program()

    x = np.ascontiguousarray(x, dtype=np.float32)
    mask = np.ascontiguousarray(mask).astype(np.uint8)
    xs = x.reshape(N_CORES, PLANES, H, W)
    ms = mask.reshape(N_CORES, PLANES, H, W)

    in_maps = [
        {"x": xs[i], "mask": ms[i], "bh": bh, "bw": bw} for i in range(N_CORES)
    ]
    res = run_bass_kernel_spmd(nc, in_maps, core_ids=list(range(N_CORES)))
    out = np.stack([res.results[i]["out"] for i in range(N_CORES)])
    return out.reshape(x.shape[0] // IMGS_PER_CORE, IMGS_PER_CORE, CHANNELS, H, W).reshape(
        -1, CHANNELS, H, W
    )
